# revision 6
# baseline (speedup 1.0000x reference)
"""2-layer GCN encoder on 8 Trainium2 NeuronCores (Bass/Tile).

Math: out = relu(Dinv (A+I) Dinv (x W) + b) twice, Dinv = deg^-1/2.
Factored as: table = (dinv * x) @ W ; agg[v] = sum_{e: dst=v} table[src_e] ;
out[v] = relu(dinv[v] * agg[v] + b)   -- no per-edge weights needed.

Distribution: dst-node sharding. Node ids padded to 100352 = 784 windows of
128. Core p owns 98 windows. Each core builds the FULL table locally from the
(replicated) layer input, then gathers + segment-sums only the edges that
point into its own windows. The inter-layer "halo exchange" (all-gather of
layer-1 activations) happens on the host between the two SPMD invocations of
the same compiled program.

Gather indices are int16 (reach 32768), so sources are split into 4 blocks
with per-block base offsets on the gather's table AP. Per (window, block) the
edge count is data-dependent while gather calls need static shapes, so the
host computes per-block caps (128-aligned) from the actual graph and pads
with repeats of block-row 0. Padded slots carry lid = -1 so their one-hot
column in S is all-zero and they contribute nothing.

Slot layout per batch of B windows (block-major so each gather call's slots
are contiguous): [blk0: w0 cap0, w1 cap0 | blk1: w0 cap1, w1 cap1 | ...].
Segment-sum on the tensor engine: per 128-slot tile, S[e, j] = (lid[e] == j)
built by the vector engine, then psum[dst, feat] += S.T @ msgs accumulated
over the window's tiles.
"""
import sys
sys.path.insert(0, "/opt/trn_rl_repo")

import math
import os
import numpy as np

N = 100000
F = 128
NCORES = 8
WIN = 128                      # dst nodes per window
NPAD = 100352                  # 784 * 128
NW = NPAD // WIN               # 784 windows
WPC = NW // NCORES             # 98 windows per core
BLOCK = 32768                  # gather idx block (int16 reach)
NBLK = 4                       # 3*32768 + 2048 = 100352
B = 2                          # windows per gather batch
NB = WPC // B                  # 49 batches

_compiled = None               # (nc, cfg) cache across invocations
_last_exec_ns = None           # filled when KERNEL_TRACE=1
_last_wall_s = None            # wall time of device calls (incl transfers)


def _wrap_idx(flat):
    """[n] -> [128, n/16] int16: slot i -> (i%16, i//16), replicated x8."""
    n = len(flat)
    m = np.asarray(flat, np.int16).reshape(n // 16, 16).T
    return np.tile(m, (8, 1))


def _host_prep(edge_index):
    """Shard edges, build per-core gather indices / lids / caps."""
    src = np.concatenate([edge_index[0], np.arange(N, dtype=np.int64)])
    dst = np.concatenate([edge_index[1], np.arange(N, dtype=np.int64)])
    deg = np.bincount(dst, minlength=NPAD).astype(np.float32)
    deg[N:] = 1.0

    g = (src // BLOCK).astype(np.int64)           # src block 0..3
    w = (dst // WIN).astype(np.int64)             # global window 0..783
    order = np.lexsort((src, g, w))               # by (window, block, src)
    src, dst, g, w = src[order], dst[order], g[order], w[order]
    lid = (dst % WIN).astype(np.float32)
    loc = src - g * BLOCK                         # in-block idx (< 32768)

    counts = np.zeros((NW, NBLK), np.int64)
    np.add.at(counts, (w, g), 1)
    caps = [int(128 * math.ceil(max(int(counts[:, b].max()), 1) / 128))
            for b in range(NBLK)]
    tw = sum(caps) // 128                         # tiles per window
    cum = np.concatenate([[0], np.cumsum(counts.reshape(-1))])  # run starts

    idxs = [np.zeros((NCORES, NB, 128, (B * caps[b]) // 16), np.int16)
            for b in range(NBLK)]
    lids = np.full((NCORES, NB, 128, B * tw), -1.0, np.float32)
    btb = np.concatenate([[0], np.cumsum([c // 128 for c in caps])])

    for c in range(NCORES):
        for b in range(NB):
            for blk in range(NBLK):
                cap = caps[blk]
                stream = np.zeros(B * cap, np.int64)
                lstream = np.full(B * cap, -1.0, np.float32)
                for r in range(B):
                    wg = (c * WPC + b * B + r) * NBLK + blk
                    s0, s1 = cum[wg], cum[wg + 1]
                    nn = s1 - s0
                    stream[r * cap : r * cap + nn] = loc[s0:s1]
                    lstream[r * cap : r * cap + nn] = lid[s0:s1]
                idxs[blk][c, b] = _wrap_idx(stream)
                # batch tile grid: block region starts at tile B*btb[blk];
                # window r owns cap/128 tiles within it
                seg = lstream.reshape(B * cap // 128, 128)
                t0 = B * btb[blk]
                lids[c, b, :, t0 : t0 + B * cap // 128] = seg.T
    cfg = {"caps": tuple(caps), "tw": int(tw),
           "btb": tuple(int(x) for x in btb)}
    data = {"idxs": idxs, "lids": lids, "degT": deg.reshape(NW, 128).T.copy()}
    return cfg, data


def _win_tiles(cfg, r):
    """Tile indices (within a batch's tile grid) owned by window r."""
    caps, btb = cfg["caps"], cfg["btb"]
    tiles = []
    for blk in range(NBLK):
        cb = caps[blk] // 128
        base = B * btb[blk] + r * cb
        tiles.extend(range(base, base + cb))
    return tiles


def _build_nc(cfg):
    from concourse import bacc, mybir
    import concourse.tile as tile
    from concourse import library_config
    import contextlib

    dt = mybir.dt
    caps, tw, btb = cfg["caps"], cfg["tw"], cfg["btb"]
    bases = [0, BLOCK, 2 * BLOCK, 3 * BLOCK]
    sizes = [BLOCK, BLOCK, BLOCK, NPAD - 3 * BLOCK]

    nc = bacc.Bacc("TRN2", target_bir_lowering=False, debug=False,
                   num_devices=NCORES)
    feat = nc.dram_tensor("feat", [NPAD, F], dt.float32, kind="ExternalInput")
    wmat = nc.dram_tensor("wmat", [F, F], dt.float32, kind="ExternalInput")
    btile = nc.dram_tensor("btile", [128, F], dt.float32, kind="ExternalInput")
    iota = nc.dram_tensor("iota", [128, 128], dt.float32, kind="ExternalInput")
    ident = nc.dram_tensor("ident", [128, 128], dt.float32, kind="ExternalInput")
    degT = nc.dram_tensor("degT", [128, NW], dt.float32, kind="ExternalInput")
    degw = nc.dram_tensor("degw", [128, WPC], dt.float32, kind="ExternalInput")
    idxt = [
        nc.dram_tensor(f"idx{b}", [NB, 128, (B * caps[b]) // 16], dt.int16,
                       kind="ExternalInput")
        for b in range(NBLK)
    ]
    lidt = nc.dram_tensor("lids", [NB, 128, B * tw], dt.float32,
                          kind="ExternalInput")
    table = nc.dram_tensor("table", [NPAD, F], dt.float32, kind="Internal")
    out = nc.dram_tensor("out", [WPC * WIN, F], dt.float32,
                         kind="ExternalOutput")

    with tile.TileContext(nc) as tc:
        ctx = contextlib.ExitStack()
        with ctx:
            cpool = ctx.enter_context(tc.tile_pool(name="const", bufs=1))
            bpool = ctx.enter_context(tc.tile_pool(name="build", bufs=3))
            mpool = ctx.enter_context(tc.tile_pool(name="msg", bufs=2))
            spool = ctx.enter_context(tc.tile_pool(name="sprep", bufs=6))
            epool = ctx.enter_context(tc.tile_pool(name="epi", bufs=3))
            pps = ctx.enter_context(tc.tile_pool(name="ps", bufs=2, space="PSUM"))

            nc.gpsimd.load_library(library_config.mlp)

            # ---- constants
            t_iota = cpool.tile([128, 128], dt.float32, tag="iota")
            nc.sync.dma_start(t_iota[:], iota.ap()[:, :])
            t_id = cpool.tile([128, 128], dt.float32, tag="ident")
            nc.sync.dma_start(t_id[:], ident.ap()[:, :])
            t_w = cpool.tile([F, F], dt.float32, tag="w")
            nc.sync.dma_start(t_w[:], wmat.ap()[:, :])
            t_b = cpool.tile([128, F], dt.float32, tag="b")
            nc.sync.dma_start(t_b[:], btile.ap()[:, :])

            t_degT = cpool.tile([128, NW], dt.float32, tag="degT")
            nc.sync.dma_start(t_degT[:], degT.ap()[:, :])
            t_dinv = cpool.tile([128, NW], dt.float32, tag="dinv")
            nc.vector.reciprocal(t_dinv[:], t_degT[:])
            nc.scalar.activation(t_dinv[:], t_dinv[:],
                                 mybir.ActivationFunctionType.Sqrt)
            t_degw = cpool.tile([128, WPC], dt.float32, tag="degw")
            nc.sync.dma_start(t_degw[:], degw.ap()[:, :])
            t_dinw = cpool.tile([128, WPC], dt.float32, tag="dinw")
            nc.vector.reciprocal(t_dinw[:], t_degw[:])
            nc.scalar.activation(t_dinw[:], t_dinw[:],
                                 mybir.ActivationFunctionType.Sqrt)

            # ---- build full table: h = (dinv * feat) @ W
            for bt in range(int(os.environ.get("KN_NWB", NW))):
                t_x = bpool.tile([128, F], dt.float32, tag="x")
                nc.sync.dma_start(t_x[:], feat.ap()[bt * 128 : (bt + 1) * 128, :])
                t_xs = bpool.tile([128, F], dt.float32, tag="xs")
                nc.vector.tensor_scalar(
                    t_xs[:], t_x[:], t_dinv[:, bt : bt + 1], None,
                    mybir.AluOpType.mult,
                )
                p_xT = pps.tile([128, 128], dt.float32, tag="xT")
                nc.tensor.transpose(p_xT[:], t_xs[:], t_id[:])
                t_xsT = bpool.tile([128, F], dt.float32, tag="xsT")
                nc.vector.tensor_copy(t_xsT[:], p_xT[:])
                p_h = pps.tile([128, F], dt.float32, tag="h")
                nc.tensor.matmul(p_h[:], t_xsT[:], t_w[:], start=True, stop=True)
                t_h = bpool.tile([128, F], dt.float32, tag="h")
                nc.vector.tensor_copy(t_h[:], p_h[:])
                nc.sync.dma_start(table.ap()[bt * 128 : (bt + 1) * 128, :], t_h[:])

            # table complete before any gather reads it
            tc.strict_bb_all_engine_barrier()

            # ---- gather + aggregate per batch of B windows
            for b in range(int(os.environ.get("KN_NB", NB))):
                t_msg = mpool.tile([128, B * tw, F], dt.float32, tag="msg")
                t_lid = spool.tile([128, B * tw], dt.float32, tag="lid")
                nc.sync.dma_start(t_lid[:], lidt.ap()[b, :, :])
                for blk in range(NBLK):
                    cap = caps[blk]
                    t_ix = spool.tile([128, (B * cap) // 16], dt.int16,
                                      tag=f"ix{blk}")
                    nc.sync.dma_start(t_ix[:], idxt[blk].ap()[b, :, :])
                    t0 = B * btb[blk]
                    nc.gpsimd.dma_gather(
                        t_msg[:, t0 : t0 + (B * cap) // 128, :],
                        table.ap()[bases[blk] : bases[blk] + sizes[blk], :],
                        t_ix[:],
                        B * cap, B * cap, F,
                        single_packet=False,
                    )
                for r in range(B):
                    k = b * B + r          # window index within core
                    p_agg = pps.tile([128, F], dt.float32, tag="agg")
                    wt = _win_tiles(cfg, r)
                    for j, t in enumerate(wt):
                        t_S = spool.tile([128, 128], dt.float32, tag="S")
                        nc.vector.tensor_scalar(
                            t_S[:], t_iota[:], t_lid[:, t : t + 1], None,
                            mybir.AluOpType.is_equal,
                        )
                        nc.tensor.matmul(
                            p_agg[:], t_S[:], t_msg[:, t, :],
                            start=(j == 0), stop=(j == len(wt) - 1),
                        )
                    t_e = epool.tile([128, F], dt.float32, tag="e")
                    nc.vector.tensor_scalar(
                        t_e[:], p_agg[:], t_dinw[:, k : k + 1], None,
                        mybir.AluOpType.mult,
                    )
                    nc.vector.tensor_tensor(
                        t_e[:], t_e[:], t_b[:], mybir.AluOpType.add
                    )
                    t_o = epool.tile([128, F], dt.float32, tag="o")
                    nc.scalar.activation(
                        t_o[:], t_e[:], mybir.ActivationFunctionType.Relu
                    )
                    nc.sync.dma_start(
                        out.ap()[k * 128 : (k + 1) * 128, :], t_o[:]
                    )

    nc.compile()
    return nc


def _run_layer(nc, data, feat_pad, W, bias):
    from concourse.bass_utils import run_bass_kernel_spmd

    iota = np.tile(np.arange(128, dtype=np.float32)[None, :], (128, 1))
    ident = np.eye(128, dtype=np.float32)
    btile = np.tile(np.asarray(bias, np.float32)[None, :], (128, 1))
    in_maps = []
    for c in range(NCORES):
        m = {
            "feat": feat_pad,
            "wmat": np.asarray(W, np.float32),
            "btile": btile,
            "iota": iota,
            "ident": ident,
            "degT": data["degT"],
            "degw": data["degT"][:, c * WPC : (c + 1) * WPC].copy(),
            "lids": data["lids"][c],
        }
        for b in range(NBLK):
            m[f"idx{b}"] = data["idxs"][b][c]
        in_maps.append(m)
    import time as _time
    trace = bool(os.environ.get("KERNEL_TRACE"))
    t0 = _time.time()
    res = run_bass_kernel_spmd(nc, in_maps, core_ids=list(range(NCORES)),
                               trace=trace)
    global _last_wall_s, _last_exec_ns
    _last_wall_s = (_last_wall_s or 0.0) + (_time.time() - t0)
    if trace:
        ns = getattr(res, "exec_time_ns", None)
        if ns:
            _last_exec_ns = (_last_exec_ns or 0) + ns
    return np.concatenate([res.results[c]["out"] for c in range(NCORES)], axis=0)


def kernel(x, edge_index, W1, b1, W2, b2):
    global _compiled
    x = np.asarray(x, np.float32)
    edge_index = np.asarray(edge_index)
    cfg, data = _host_prep(edge_index)
    if _compiled is None or _compiled[1] != cfg:
        _compiled = (_build_nc(cfg), cfg)
    nc = _compiled[0]

    xpad = np.zeros((NPAD, F), np.float32)
    xpad[:N] = x
    out1 = _run_layer(nc, data, xpad, W1, b1)        # [NPAD, F] relu'd
    h1 = np.zeros((NPAD, F), np.float32)
    h1[:N] = out1[:N]
    out2 = _run_layer(nc, data, h1, W2, b2)
    return out2[:N].astype(np.float32)
